# revision 32
# baseline (speedup 1.0000x reference)
"""Trainium2 Bass/Tile kernel for EnrichedGeometricEmbedding.

Full-input contract: kernel(**inputs) takes the complete tensors, shards the
batch dim across 8 NeuronCores (B=8 -> 1 batch row per core), runs one SPMD
program via run_bass_kernel_spmd, and gathers the full [8, 1024, 32, 384]
output. Memory-bound: the 50 MB/core output write (~140 us at the modeled
~360 GB/s) sets the roofline; the kernel packs output DMAs back-to-back and
shrinks the prologue before the first output DMA.

Differences vs the earlier 183 us version:
  * Inputs load as 3072-B-contiguous runs (partition p holds groups
    p*8..p*8+7): halves input DMA time. Column order c = t*128 + p is kept
    through the whole pipeline; the group permutation g = p*8 + t appears
    only in the input/output DRAM access patterns.
  * The per-group smallest-eigenvalue chain runs entirely on DVE: bit-trick
    rsqrt seed + one order-2 Householder step, and
    sin(acos(r)/3 + pi/6) = 1/2 + P(r) + sqrt(1-r)*Q(r) with quadratic P, Q
    (max |h err| ~1e-6, curv err < ~1e-3). No ACT sqrt/trig tables ->
    zero LoadActFuncSet on the critical path (one exp_and_others load at t~0
    covers Square/Exp/Abs).
  * Half-split pipeline: groups 0..511 (xdkT tiles 0-3) go through
    transpose -> lap/g42/curv -> flo -> matmul -> output first; tiles 4-7
    transposes, their copies and half-1 lap/g42 work are interleaved into
    the half-0 main loop, riding each engine's slack (PE ~480 ns, ACT ~950
    ns per 2185 ns output DMA).
  * One [128,24] mean transpose in (d,t) column order makes the m3 rows
    partition-contiguous (no strided DMAs -- those silently corrupt data).

Main loop per (half, quarter-phase, k): rbf rows via ACT Square(bias=-c) +
Exp(scale=-2) on a PE-broadcast of x (ebig selection matmul); one K=128 and
one K=6 accumulating fp32r matmul per 128-point tile; PSUM->SBUF copies
alternate DVE/ACT; one HWDGE DMA per 512 points scatters [128, 4, 384] rows
to DRAM. Accumulation stays fp32, ~3e-4 relative error.
"""

import math

import numpy as np

B, S, K, D = 8, 1024, 32, 3
F = 43                      # FEAT_DIM
OUT = 384
G = S                       # groups per core
P = S * K                   # points per core (32768)
NT = G // 128               # group tiles (8)
TOTAL = F * D + 1 + D       # 133
HK = K // 4                 # k per phase
HG = G // 2                 # groups per half (512)

# h(r) = sin(acos(r)/3 + pi/6) ~= 1/2 + P(r) + sqrt(1-r)*Q(r)
PC = (-0.05987054109573364, 0.06474298238754272, -0.004871369805186987)
QC = (0.42589694261550903, -0.01846284233033657, 0.0007540023070760071)
MAGIC = 0x5F3759DF

NBA = 128 + 96 + 1          # ident | rmat | negc
NBB = OUT + OUT             # whi | wlo6

_prog_cache = {}


def _build_program():
    import concourse.bacc as bacc
    import concourse.mybir as mybir
    from concourse.tile import TileContext

    DT = mybir.dt.float32
    DTR = mybir.dt.float32r
    IT = mybir.dt.int32
    Act = mybir.ActivationFunctionType
    Op = mybir.AluOpType
    X = mybir.AxisListType.X

    C = np.linspace(-1.0, 1.0, F + 2, dtype=np.float64)[1:-1]
    C42 = float(C[F - 1])

    nc = bacc.Bacc("TRN2", target_bir_lowering=False, debug=False, num_devices=8)
    xyz_d = nc.dram_tensor("xyz", [P, D], DT, kind="ExternalInput").ap()
    nbr_d = nc.dram_tensor("nbr", [P, D], DT, kind="ExternalInput").ap()
    blobA_d = nc.dram_tensor("blobA", [128, NBA], DTR, kind="ExternalInput").ap()
    blobB_d = nc.dram_tensor("blobB", [128, NBB], DTR, kind="ExternalInput").ap()
    ebig_d = nc.dram_tensor("ebig", [96, K * 128], DTR, kind="ExternalInput").ap()
    out_d = nc.dram_tensor("out", [P, OUT], DT, kind="ExternalOutput").ap()

    def view_ti(t24, width, i):
        return t24.rearrange("p (t i) -> p i t", i=width)[:, i : i + 1, :].squeeze(1)

    with TileContext(nc) as tc:
        with (
            tc.tile_pool(name="const", bufs=1) as constp,
            tc.tile_pool(name="stats", bufs=1) as statp,
            tc.tile_pool(name="gwork", bufs=1) as gwp,
            tc.tile_pool(name="flopool", bufs=1) as flop,
            tc.tile_pool(name="main", bufs=6) as mainp,
        ):
            tpsum = tc.alloc_tile_pool(name="tpsum", bufs=1, space="PSUM")
            ppsum = tc.alloc_tile_pool(name="ppsum", bufs=1, space="PSUM")

            # ---- loads ----
            blobA = constp.tile([128, NBA], DTR)
            nc.sync.dma_start(blobA[:], blobA_d[:])
            ident = blobA[:, 0:128].bitcast(DT)
            identr = blobA[:, 0:128]
            rmat = blobA[0:D, 128:224]
            negc = blobA[:, 224:225].bitcast(DT)

            n_all = gwp.tile([128, NT * K * D], DT, tag="nall")
            nc.sync.dma_start(
                n_all.rearrange("p (t f) -> p t f", f=K * D),
                nbr_d.rearrange("(p t k) d -> p t (k d)", p=128, t=NT),
            )
            x_all = gwp.tile([128, NT * K * D], DTR, tag="xall")
            xsrc = xyz_d.rearrange("(p u t k) d -> u p t (k d)", p=128, u=2, t=NT // 2)
            for hx in range(2):
                nc.scalar.dma_start(
                    x_all[:, hx * 384 : (hx + 1) * 384].rearrange(
                        "p (t f) -> p t f", f=K * D
                    ),
                    xsrc[hx : hx + 1, :, :, :].squeeze(0).bitcast(DTR),
                )
            blobB = constp.tile([128, NBB], DTR)
            nc.scalar.dma_start(blobB[:], blobB_d[:])
            whi = blobB[:, 0:OUT]
            wlo = blobB[0:6, OUT : OUT + OUT]
            ebig = constp.tile([96, K * 128], DTR)
            for qe in range(4):
                nc.scalar.dma_start(
                    ebig[:, qe * 1024 : (qe + 1) * 1024],
                    ebig_d[:, qe * 1024 : (qe + 1) * 1024],
                )

            # warm the ACT table (exp_and_others covers Square/Exp/Abs)
            tiny = constp.tile([1, 1], DT)
            nc.vector.memset(tiny[:], 0.0)
            nc.scalar.activation(tiny[:], tiny[:], Act.Exp)

            c242_t = constp.tile([96, 1], DT)
            nc.vector.memset(c242_t[:], -2.0 * C42)
            ce42_t = constp.tile([96, 1], DT)
            nc.vector.memset(ce42_t[:], -2.0 * C42 * C42)
            ones_t = constp.tile([128, HK * HG // 128], DT)
            nc.vector.memset(ones_t[:], 1.0)

            # ---- transposes: per (t, d) [128,32] -> psum [96,128] at row
            # d*32, one [96,128] ACT copy per tile; tiles 0-3 now, 4-7 are
            # emitted inside the half-0 main loop ----
            xdkT = statp.tile([96, G], DTR)

            def do_tile(t):
                x_kd = x_all[:, t * K * D : (t + 1) * K * D].rearrange(
                    "g (k d) -> g d k", d=D
                )
                for d in range(D):
                    xps = tpsum.tile(
                        [K, 128], DT, tag="tp", bufs=2, name=f"xps{t}_{d}"
                    )
                    nc.tensor.transpose(
                        xps[:], x_kd[:, d : d + 1, :].squeeze(1).bitcast(DT), ident
                    )
                    nc.scalar.copy(
                        xdkT[d * K : (d + 1) * K, t * 128 : (t + 1) * 128], xps[:]
                    )

            for t in range(2):
                do_tile(t)

            # ---- stats on DVE ----
            n_v = n_all.rearrange("p (t k d) -> p t d k", k=K, d=D)
            n4 = n_all.rearrange("p (t k d) -> p t k d", k=K, d=D)
            m_all = statp.tile([128, NT * D], DT)  # cols (d, t)
            m_v = m_all.rearrange("p (d t) -> p t d", t=NT)
            nc.vector.tensor_reduce(m_v, n_v, axis=X, op=Op.add)
            nc.vector.tensor_scalar_mul(m_all[:], m_all[:], 1.0 / K)
            nc.vector.tensor_sub(
                n_v, n_v, m_v.unsqueeze(3).broadcast_to([128, NT, D, K])
            )

            # mean transpose: [128, 24] -> [24, 128] rows (d*8+t)
            mps = ppsum.tile([NT * D, 128], DT, tag="mp", bufs=1)
            nc.tensor.transpose(mps[:], m_all[:], ident)
            do_tile(2)
            mT = statp.tile([NT * D, 128], DT)
            nc.vector.tensor_copy(mT[:], mps[:])
            m3 = statp.tile([D, G], DTR)
            for d in range(D):
                nc.scalar.dma_start(
                    m3[d : d + 1, :].rearrange("o (t g) -> o t g", g=128),
                    mT[d * NT : (d + 1) * NT, :].bitcast(DTR),
                )

            # covariance sums
            o2 = gwp.tile([128, NT * K * 2], DT, tag="o2")
            nc.vector.tensor_mul(
                o2.rearrange("p (t k i) -> p t k i", k=K, i=2),
                n4[:, :, :, 0:2],
                n4[:, :, :, 1:3],
            )
            o1 = gwp.tile([128, NT * K], DT, tag="o1")
            nc.vector.tensor_mul(
                o1.rearrange("p (t k) -> p t k", k=K),
                n4[:, :, :, 0:1].squeeze(3),
                n4[:, :, :, 2:3].squeeze(3),
            )
            nc.vector.tensor_mul(n_all[:], n_all[:], n_all[:])  # squares in place
            Ud = statp.tile([128, NT * D], DT)
            nc.vector.tensor_reduce(
                Ud.rearrange("p (t d) -> p t d", d=D),
                n_all.rearrange("p (t k d) -> p t d k", k=K, d=D),
                axis=X,
                op=Op.add,
            )
            Uo2 = statp.tile([128, NT * 2], DT)
            nc.vector.tensor_reduce(
                Uo2.rearrange("p (t i) -> p t i", i=2),
                o2.rearrange("p (t k i) -> p t i k", k=K, i=2),
                axis=X,
                op=Op.add,
            )
            Uo1 = statp.tile([128, NT], DT)
            nc.vector.tensor_reduce(
                Uo1[:], o1.rearrange("p (t k) -> p t k", k=K), axis=X, op=Op.add
            )
            do_tile(3)

            # ---- eigen chain, all [128, NT] on DVE ----
            def nv(name, w=NT):
                return statp.tile([128, w], DT, tag=name, name=name)

            d0 = view_ti(Ud, 3, 0)
            d1 = view_ti(Ud, 3, 1)
            d2 = view_ti(Ud, 3, 2)
            f_v = view_ti(Uo2, 2, 0)
            g_v = view_ti(Uo2, 2, 1)
            h_v = Uo1[:]

            tr = nv("tr")
            nc.vector.tensor_reduce(
                tr[:], Ud.rearrange("p (t d) -> p t d", d=D), axis=X, op=Op.add
            )
            den = nv("den")
            nc.vector.tensor_scalar_add(den[:], tr[:], (K - 1) * 1e-6)
            rtr = nv("rtr")
            nc.vector.reciprocal(rtr[:], den[:])
            q_t = nv("q")
            nc.vector.tensor_scalar_mul(q_t[:], tr[:], 1.0 / 3.0)
            dd = statp.tile([128, NT * D], DT)
            nc.vector.scalar_tensor_tensor(
                dd.rearrange("p (t i) -> p t i", i=D),
                tr.unsqueeze(2).broadcast_to([128, NT, 3]),
                -1.0 / 3.0,
                Ud.rearrange("p (t d) -> p t d", d=D),
                op0=Op.mult,
                op1=Op.add,
            )
            ddsq = statp.tile([128, NT * D], DT)
            nc.vector.tensor_mul(ddsq[:], dd[:], dd[:])
            ddred = nv("ddred")
            nc.vector.tensor_reduce(
                ddred[:], ddsq.rearrange("p (t i) -> p t i", i=D), axis=X, op=Op.add
            )
            o2sq = statp.tile([128, NT * 2], DT)
            nc.vector.tensor_mul(o2sq[:], Uo2[:], Uo2[:])
            o1sq = nv("o1sq")
            nc.vector.tensor_mul(o1sq[:], Uo1[:], Uo1[:])
            osr = nv("osr")
            nc.vector.tensor_reduce(
                osr[:], o2sq.rearrange("p (t i) -> p t i", i=2), axis=X, op=Op.add
            )
            osum = nv("osum")
            nc.vector.tensor_add(osum[:], osr[:], o1sq[:])
            p2 = nv("p2")
            nc.vector.scalar_tensor_tensor(
                p2[:], osum[:], 2.0, ddred[:], op0=Op.mult, op1=Op.add
            )
            p2c = nv("p2c")
            nc.vector.tensor_scalar_max(p2c[:], p2[:], 1e-25)

            dd0 = view_ti(dd, 3, 0)
            dd1 = view_ti(dd, 3, 1)
            dd2 = view_ti(dd, 3, 2)

            def rsqrt_h2(xin, pref):
                ish = statp.tile([128, NT], IT, tag=pref + "i", name=pref + "i")
                nc.vector.tensor_scalar(
                    ish[:], xin.bitcast(IT), 1, None, op0=Op.logical_shift_right
                )
                y0i = statp.tile([128, NT], IT, tag=pref + "s", name=pref + "s")
                nc.vector.tensor_scalar(
                    y0i[:], ish[:], -1, MAGIC, op0=Op.mult, op1=Op.add
                )
                y0 = y0i.bitcast(DT)
                a = nv(pref + "a")
                nc.vector.tensor_mul(a[:], y0, y0)
                bb = nv(pref + "b")
                nc.vector.tensor_mul(bb[:], xin, a[:])
                cc = nv(pref + "c")
                nc.vector.tensor_mul(cc[:], bb[:], bb[:])
                ee = nv(pref + "e")
                nc.vector.tensor_scalar(
                    ee[:], bb[:], -1.25, 1.875, op0=Op.mult, op1=Op.add
                )
                ff = nv(pref + "f")
                nc.vector.scalar_tensor_tensor(
                    ff[:], cc[:], 0.375, ee[:], op0=Op.mult, op1=Op.add
                )
                yy = nv(pref + "y")
                nc.vector.tensor_mul(yy[:], y0, ff[:])
                return yy

            y = rsqrt_h2(p2c[:], "r1")
            p_t = nv("p")
            nc.vector.scalar_tensor_tensor(
                p_t[:], p2c[:], 1.0 / math.sqrt(6.0), y[:], op0=Op.mult, op1=Op.mult
            )
            qp = nv("qp")
            nc.vector.tensor_sub(qp[:], q_t[:], p_t[:])
            y2 = nv("y2")
            nc.vector.tensor_mul(y2[:], y[:], y[:])
            y3 = nv("y3")
            nc.vector.tensor_mul(y3[:], y2[:], y[:])

            det = nv("det")
            scr = nv("scr")
            nc.vector.tensor_mul(det[:], dd0, dd1)
            nc.vector.tensor_mul(det[:], det[:], dd2)
            nc.vector.tensor_mul(scr[:], f_v, g_v)
            nc.vector.scalar_tensor_tensor(
                scr[:], scr[:], 2.0, h_v, op0=Op.mult, op1=Op.mult
            )
            nc.vector.tensor_add(det[:], det[:], scr[:])
            nc.vector.tensor_mul(scr[:], dd0, view_ti(o2sq, 2, 1))
            nc.vector.tensor_sub(det[:], det[:], scr[:])
            nc.vector.tensor_mul(scr[:], dd1, o1sq[:])
            nc.vector.tensor_sub(det[:], det[:], scr[:])
            nc.vector.tensor_mul(scr[:], dd2, view_ti(o2sq, 2, 0))
            nc.vector.tensor_sub(det[:], det[:], scr[:])

            r_t = nv("r")
            nc.vector.scalar_tensor_tensor(
                r_t[:], det[:], 0.5 * 6.0**1.5, y3[:], op0=Op.mult, op1=Op.mult
            )
            nc.vector.tensor_scalar(
                r_t[:], r_t[:], 0.999999, -0.999999, op0=Op.min, op1=Op.max
            )
            wv = nv("wv")
            nc.vector.tensor_scalar(wv[:], r_t[:], -1.0, 1.0, op0=Op.mult, op1=Op.add)
            z = rsqrt_h2(wv[:], "r2")
            sqw = nv("sqw")
            nc.vector.tensor_mul(sqw[:], wv[:], z[:])
            Pv = nv("Pv")
            nc.vector.tensor_scalar(Pv[:], r_t[:], PC[2], PC[1], op0=Op.mult, op1=Op.add)
            nc.vector.tensor_mul(Pv[:], Pv[:], r_t[:])
            nc.vector.tensor_scalar_add(Pv[:], Pv[:], PC[0])
            Qv = nv("Qv")
            nc.vector.tensor_scalar(Qv[:], r_t[:], QC[2], QC[1], op0=Op.mult, op1=Op.add)
            nc.vector.tensor_mul(Qv[:], Qv[:], r_t[:])
            nc.vector.tensor_scalar_add(Qv[:], Qv[:], QC[0])
            hm = nv("hm")
            nc.vector.tensor_mul(hm[:], sqw[:], Qv[:])
            nc.vector.tensor_add(hm[:], hm[:], Pv[:])
            tt = nv("tt")
            nc.vector.tensor_mul(tt[:], p_t[:], hm[:])
            lam = nv("lam")
            nc.vector.scalar_tensor_tensor(
                lam[:], tt[:], -2.0, qp[:], op0=Op.mult, op1=Op.add
            )
            curv_all = nv("curv")
            nc.vector.tensor_mul(curv_all[:], lam[:], rtr[:])

            # curv row -> [1, G]
            cps = ppsum.tile([NT, 128], DT, tag="cp", bufs=1)
            nc.tensor.transpose(cps[:], curv_all[:], ident)
            ctv = statp.tile([NT, 128], DT)
            nc.vector.tensor_copy(ctv[:], cps[:])
            curv_g = statp.tile([1, G], DT)
            nc.scalar.dma_start(curv_g.rearrange("o (t g) -> o t g", g=128), ctv[:])

            lapT = statp.tile([96, G], DT)
            g42f = statp.tile([96, G], DT)

            def half_prep(half):
                sl = slice(half * HG, (half + 1) * HG)
                pool = ppsum if half == 0 else tpsum
                mrepp = pool.tile(
                    [96, HG], DT, tag=f"mr{half}", bufs=1, name=f"mrep{half}"
                )
                nc.tensor.matmul(mrepp[:], rmat, m3[:, sl], start=True, stop=True)
                nc.vector.scalar_tensor_tensor(
                    g42f[:, sl],
                    xdkT[:, sl].bitcast(DT),
                    c242_t[:],
                    xdkT[:, sl].bitcast(DT),
                    op0=Op.add,
                    op1=Op.mult,
                )
                nc.scalar.activation(
                    g42f[:, sl], g42f[:, sl], Act.Exp, bias=ce42_t[:], scale=-2.0
                )
                nc.vector.tensor_sub(
                    lapT[:, sl], xdkT[:, sl].bitcast(DT), mrepp[:]
                )
                nc.scalar.activation(lapT[:, sl], lapT[:, sl], Act.Abs)

            half_prep(0)

            ppsum.release()
            xbp = tc.alloc_tile_pool(name="xbpsum", bufs=1, space="PSUM")
            outp = tc.alloc_tile_pool(name="outpsum", bufs=2, space="PSUM")

            outv = out_d.rearrange("(p h c k) x -> k h p c x", p=128, h=2, c=4, k=K)

            # deferred half-1 work, emitted between half-0 main-loop tiles
            deferred = [lambda t=t: do_tile(t) for t in range(4, 8)]
            deferred.append(lambda: half_prep(1))

            for half in range(2):
                csl = slice(half * HG, (half + 1) * HG)
                for phase in range(4):
                    k0 = phase * HK
                    flo = flop.tile([6, HK * HG], DTR, tag="flo", bufs=2)
                    nc.scalar.dma_start(
                        flo[0:1, :].rearrange("o (k g) -> o k g", g=HG),
                        g42f[2 * K + k0 : 2 * K + k0 + HK, csl].bitcast(DTR),
                    )
                    nc.scalar.dma_start(
                        flo[1:2, :].rearrange("o (k g) -> o k g", g=HG),
                        curv_g[:, csl]
                        .bitcast(DTR)
                        .unsqueeze(1)
                        .broadcast_to([1, HK, HG]),
                    )
                    for d in range(D):
                        nc.scalar.dma_start(
                            flo[2 + d : 3 + d, :].rearrange("o (k g) -> o k g", g=HG),
                            lapT[d * K + k0 : d * K + k0 + HK, csl].bitcast(DTR),
                        )
                    if half == 0 and phase < 2:
                        nc.scalar.dma_start(
                            flo[5:6, :].rearrange(
                                "o (a b) -> o a b", b=HK * HG // 128
                            ),
                            ones_t.bitcast(DTR),
                        )
                    for k in range(k0, k0 + HK):
                        if half == 0 and phase > 0 and deferred:
                            deferred.pop(0)()
                        xb = xbp.tile([128, HG], DT, tag="xb")
                        nc.tensor.matmul(
                            xb[:],
                            ebig[:, k * 128 : (k + 1) * 128],
                            xdkT[:, csl],
                            start=True,
                            stop=True,
                        )
                        t2 = mainp.tile([128, HG], DT, tag="t2")
                        nc.scalar.activation(t2[:], xb[:], Act.Square, bias=negc)
                        fhi = mainp.tile([128, HG], DTR, tag="fhi")
                        nc.scalar.activation(fhi[:], t2[:], Act.Exp, scale=-2.0)
                        so = mainp.tile([128, 4 * OUT], DT, tag="so", bufs=6)
                        for pair in range(2):
                            ps = outp.tile([128, 1024], DT, tag="ps")
                            for c in range(2):
                                j = pair * 2 + c
                                nc.tensor.matmul(
                                    ps[:, c * 512 : c * 512 + OUT],
                                    fhi[:, j * 128 : (j + 1) * 128],
                                    whi,
                                    start=True,
                                    stop=False,
                                )
                                lo = (k - k0) * HG + j * 128
                                nc.tensor.matmul(
                                    ps[:, c * 512 : c * 512 + OUT],
                                    flo[0:6, lo : lo + 128],
                                    wlo,
                                    start=False,
                                    stop=True,
                                )
                            ps_v = ps.rearrange("p (c x) -> p c x", x=512)[:, :, 0:OUT]
                            so_v = so.rearrange("p (c x) -> p c x", x=OUT)[
                                :, pair * 2 : pair * 2 + 2, :
                            ]
                            cnt = (k * 2 + half) * 2 + pair
                            if cnt % 2 == 1:
                                nc.scalar.copy(so_v, ps_v)
                            else:
                                nc.vector.tensor_copy(so_v, ps_v)
                        out_ap = (
                            outv[k : k + 1, half : half + 1, :, :, :]
                            .squeeze(0)
                            .squeeze(0)
                        )
                        nc.sync.dma_start(
                            out_ap, so.rearrange("p (c x) -> p c x", x=OUT)
                        )
            outp.release()
            xbp.release()
            tpsum.release()

    nc.compile()
    return nc


def _get_program():
    if "nc" not in _prog_cache:
        _prog_cache["nc"] = _build_program()
    return _prog_cache["nc"]


def kernel(xyz, neighbor_xyz, projection_weight, projection_bias):
    from concourse.bass_utils import run_bass_kernel_spmd

    nc = _get_program()

    w = np.ascontiguousarray(projection_weight, dtype=np.float32)
    bias = np.ascontiguousarray(projection_bias, dtype=np.float32)
    whi = np.ascontiguousarray(w[:128])
    wlo6 = np.concatenate([w[128:TOTAL], bias[None, :]], axis=0).astype(np.float32)

    ident = np.eye(128, dtype=np.float32)
    rmat = np.zeros((128, 96), dtype=np.float32)
    for d in range(D):
        rmat[d, d * K : (d + 1) * K] = 1.0
    c = np.linspace(-1.0, 1.0, F + 2, dtype=np.float32)[1:-1]
    negc = (-c[np.arange(128) % F]).reshape(128, 1).astype(np.float32)

    blobA = np.zeros((128, NBA), dtype=np.float32)
    blobA[:, 0:128] = ident
    blobA[:, 128:224] = rmat
    blobA[:, 224:225] = negc
    blobB = np.zeros((128, NBB), dtype=np.float32)
    blobB[:, 0:OUT] = whi
    blobB[0:6, OUT:] = wlo6
    ebig = np.zeros((96, K * 128), dtype=np.float32)
    for k in range(K):
        for r in range(128):
            ebig[(r // F) * K + k, k * 128 + r] = 1.0

    xyz = np.ascontiguousarray(xyz, dtype=np.float32)
    nbr = np.ascontiguousarray(neighbor_xyz, dtype=np.float32)
    in_maps = []
    for core in range(B):
        in_maps.append(
            {
                "xyz": xyz[core].reshape(P, D),
                "nbr": nbr[core].reshape(P, D),
                "blobA": blobA,
                "blobB": blobB,
                "ebig": ebig,
            }
        )
    res = run_bass_kernel_spmd(nc, in_maps, list(range(B)))
    out = np.stack(
        [res.results[i]["out"].reshape(S, K, OUT) for i in range(B)], axis=0
    )
    return out
